# revision 17
# baseline (speedup 1.0000x reference)
"""Trainium2 Bass kernel for nn_Aggregator (GNN message passing).

Computation per (batch b, iter i):
    scores[s] = mean_d(ue[b,d] * nr[b,i,s,d])          s in [0,32)
    w = softmax_s(scores)
    out[b,i,:] = relu(mean_s(w[s] * nv[b,i,s,:]))

Sharding: pure data parallel over the batch axis, 4096 / 8 cores = 512
batches per core.  Each core runs an identical single-core program.

Per-core structure (512 batches = 4 groups of 128):
  Phase A (scores, batches on partitions):
    - NR tile [128, 32, 64] per (group, iter): contiguous 8KB/partition DMA
    - prod = NR * broadcast(UE)      (DVE tensor_tensor, stride-0 AP)
    - sc[:, i, :] = reduce_sum_d     (DVE segmented reduce, axis=X)
    - e = exp(sc / 64)               (ACT, one op per group)
    - esum = reduce_sum_s, rinv = 1/esum, w = e * broadcast(rinv)
  Phase B (aggregation on TensorE, neighbors on partitions):
    - wT = PE-transpose of w halves -> [(l,s) partitions, batch free]
    - block-diagonal lhsT tiles [128, 128] (pre-zeroed once, strided fills)
    - per half-batch (b, h): matmul(out=psum[4g:4g+4,:], lhsT=eblk[:,4g:4g+4],
      rhs=nv_tile[128,64]); 32 matmuls fill one [128, 64] PSUM tile
    - relu(psum / 32) on ACT evacuates to SBUF, one 32KB contiguous DMA out
"""

import numpy as np

import concourse.bacc as bacc
import concourse.mybir as mybir
import concourse.tile as tile
from concourse.masks import make_identity

B_FULL = 4096
NITER = 8
NSIZE = 32
DIM = 64
N_CORES = 8
B_CORE = B_FULL // N_CORES  # 512


def build_nc(bc=B_CORE, reps=1):
    """Build + compile the per-core Bass program for bc batches.

    reps > 1 unrolls the whole body N times inside one NEFF (same inputs,
    same outputs) -- used only for slope-based wall-clock timing."""
    assert bc % 128 == 0
    ngroups = bc // 128

    nc = bacc.Bacc("TRN2", target_bir_lowering=False, debug=False)

    nv = nc.dram_tensor(
        "neighbor_vectors", [bc, NITER * NSIZE, DIM], mybir.dt.float32,
        kind="ExternalInput",
    )
    nr = nc.dram_tensor(
        "neighbor_relations", [bc, NITER * NSIZE, DIM], mybir.dt.float32,
        kind="ExternalInput",
    )
    ue = nc.dram_tensor(
        "user_embeddings", [bc, DIM], mybir.dt.float32, kind="ExternalInput"
    )
    out = nc.dram_tensor(
        "out", [bc, NITER, DIM], mybir.dt.float32, kind="ExternalOutput"
    )

    with tile.TileContext(nc) as tc:
        with (
            tc.tile_pool(name="singles", bufs=1) as singles,
            tc.tile_pool(name="nrp", bufs=4) as nrp,
            tc.tile_pool(name="prodp", bufs=2) as prodp,
            tc.tile_pool(name="scp", bufs=2) as scp,
            tc.tile_pool(name="smallp", bufs=4) as smallp,
            tc.tile_pool(name="wtp", bufs=4) as wtp,
            tc.tile_pool(name="uep", bufs=2) as uep,
            tc.tile_pool(name="nvp", bufs=4) as nvp,
            tc.tile_pool(name="eblkp", bufs=3) as eblkp,
            tc.tile_pool(name="outp", bufs=4) as outp,
            tc.tile_pool(name="psmm", bufs=4, space="PSUM") as psmm,
            tc.tile_pool(name="pstr", bufs=2, space="PSUM") as pstr,
        ):
            ident = singles.tile([128, 128], mybir.dt.float32)
            make_identity(nc, ident)

            # Block-diag mask for the lhsT scatter:
            # mask[p, c] = ((p % 64)//16 == c), built once from the identity.
            # Full 128 partitions so slices at base partition 0 and 64 both
            # exist (walrus requires equal SBUF base partitions for the two
            # inputs of TensorTensor).
            mask = singles.tile([128, 4], mybir.dt.float32)
            nc.vector.reduce_sum(
                mask[:, :],
                ident[:, 0:128].rearrange("p (r c k) -> p c r k", r=2, k=16),
                axis=mybir.AxisListType.XY,
            )

            for rep in range(reps):
              for g in range(ngroups):
                b0 = g * 128

                ue_t = uep.tile([128, DIM], mybir.dt.float32)
                nc.sync.dma_start(out=ue_t[:, :], in_=ue[b0:b0 + 128, :])

                sc = scp.tile([128, NITER, NSIZE], mybir.dt.float32)
                for i in range(NITER):
                    nr_t = nrp.tile([128, NSIZE, DIM], mybir.dt.float32)
                    nc.sync.dma_start(
                        out=nr_t[:, :, :],
                        in_=nr[b0:b0 + 128, i * NSIZE:(i + 1) * NSIZE, :],
                    )
                    prod = prodp.tile([128, NSIZE, DIM], mybir.dt.float32)
                    nc.vector.tensor_mul(
                        prod[:, :, :],
                        nr_t[:, :, :],
                        ue_t[:, :].unsqueeze(1).to_broadcast((128, NSIZE, DIM)),
                    )
                    nc.vector.reduce_sum(
                        sc[:, i, :], prod[:, :, :], axis=mybir.AxisListType.X
                    )

                e_t = scp.tile([128, NITER, NSIZE], mybir.dt.float32)
                nc.scalar.activation(
                    e_t[:, :, :], sc[:, :, :],
                    mybir.ActivationFunctionType.Exp, scale=1.0 / DIM,
                )
                es = smallp.tile([128, NITER], mybir.dt.float32)
                nc.vector.reduce_sum(
                    es[:, :], e_t[:, :, :], axis=mybir.AxisListType.X
                )
                rinv = smallp.tile([128, NITER], mybir.dt.float32)
                nc.vector.reciprocal(rinv[:, :], es[:, :])
                w_t = scp.tile([128, NITER, NSIZE], mybir.dt.float32)
                nc.vector.tensor_mul(
                    w_t[:, :, :],
                    e_t[:, :, :],
                    rinv[:, :].unsqueeze(2).to_broadcast((128, NITER, NSIZE)),
                )
                w_flat = w_t.rearrange("p i s -> p (i s)")

                # Row-parity permutation (walrus requires 2D matmul-weight
                # APs, so materialize the permuted order with a DVE copy),
                # then transpose: wT partition r' = 64j + p holds
                # w[b, row 2p+j] of half h.
                wT = []
                for h in range(2):
                    wperm = wtp.tile([128, 128], mybir.dt.float32,
                                     name="wperm", tag="wperm")
                    nc.vector.tensor_copy(
                        wperm.rearrange("b (j l p2) -> b j l p2", j=2, l=4),
                        w_flat[:, h * 128:(h + 1) * 128].rearrange(
                            "b (l p2 j) -> b j l p2", l=4, j=2),
                    )
                    ps = pstr.tile([128, 128], mybir.dt.float32)
                    nc.tensor.transpose(ps[:, :], wperm[:, :], ident[:, :])
                    wt_sb = wtp.tile([128, 128], mybir.dt.float32, tag="wt_sb")
                    nc.scalar.copy(wt_sb[:, :], ps[:, :])
                    wT.append(wt_sb)

                for h in range(2):
                    for bb in range(4):  # 32-batch blocks (one supertile)
                        # Block-diagonal lhsT per row parity j:
                        # eblk_j[p, q, c] = w[b_q, 4h+c, 2(p-16c)+j] for
                        # p//16 == c, else 0 == broadcast(wT) * mask.
                        ebs = []
                        for j in range(2):
                            ebj = eblkp.tile([64, 32, 4], mybir.dt.float32,
                                             name=f"ebj{j}", tag=f"ebj{j}")
                            nc.gpsimd.tensor_mul(
                                ebj[:, :, :],
                                wT[h][64 * j:64 * j + 64,
                                      bb * 32:(bb + 1) * 32]
                                .unsqueeze(2).to_broadcast((64, 32, 4)),
                                mask[64 * j:64 * j + 64, :].unsqueeze(1)
                                .to_broadcast((64, 32, 4)),
                            )
                            ebs.append(ebj)
                        # One 1MB DMA per supertile; partition p holds two
                        # consecutive neighbor rows (2p, 2p+1) = 512B elems.
                        nvst = nvp.tile([64, 32, 2, DIM], mybir.dt.float32)
                        nc.scalar.dma_start(
                            out=nvst[:, :, :, :],
                            in_=nv[b0 + bb * 32:b0 + bb * 32 + 32,
                                   h * 128:(h + 1) * 128, :].rearrange(
                                       "g (p j) d -> p g j d", j=2),
                        )
                        # Flipped matmul: out[d, 4g+l] += sum_p nv * eblk_j
                        # (PE can't write PSUM at partition offset 4g, but
                        # free offsets are unconstrained -> accumulate along
                        # free dim, transpose back at the end.)
                        pmm = psmm.tile([DIM, 128], mybir.dt.float32)
                        for lg in range(32):
                            for j in range(2):
                                nc.tensor.matmul(
                                    pmm[:, 4 * lg:4 * lg + 4],
                                    lhsT=nvst[:, lg, j, :],
                                    rhs=ebs[j][:, lg, :],
                                    start=(j == 0), stop=(j == 1),
                                )
                        agg_sb = outp.tile([DIM, 128], mybir.dt.float32,
                                           tag="agg_sb")
                        nc.scalar.copy(agg_sb[:, :], pmm[:, :])
                        psT = pstr.tile([128, DIM], mybir.dt.float32,
                                        tag="psT")
                        nc.tensor.transpose(
                            psT[:, :], agg_sb[:, :], ident[0:DIM, 0:DIM]
                        )
                        osb = outp.tile([128, DIM], mybir.dt.float32)
                        nc.scalar.activation(
                            osb[:, :], psT[:, :],
                            mybir.ActivationFunctionType.Relu,
                            scale=1.0 / NSIZE,
                        )
                        nc.scalar.dma_start(
                            out=out[b0 + bb * 32:b0 + bb * 32 + 32,
                                    4 * h:4 * h + 4, :],
                            in_=osb[:, :],
                        )

    nc.compile()
    return nc


_NC_CACHE = {}


def _get_nc(bc=B_CORE):
    if bc not in _NC_CACHE:
        _NC_CACHE[bc] = build_nc(bc)
    return _NC_CACHE[bc]


def _shard_inputs(neighbor_vectors, neighbor_relations, user_embeddings):
    nv = np.ascontiguousarray(np.asarray(neighbor_vectors, dtype=np.float32))
    nr = np.ascontiguousarray(np.asarray(neighbor_relations, dtype=np.float32))
    ue = np.ascontiguousarray(np.asarray(user_embeddings, dtype=np.float32))
    in_maps = []
    for c in range(N_CORES):
        sl = slice(c * B_CORE, (c + 1) * B_CORE)
        in_maps.append({
            "neighbor_vectors": np.ascontiguousarray(nv[sl]),
            "neighbor_relations": np.ascontiguousarray(nr[sl]),
            "user_embeddings": np.ascontiguousarray(ue[sl]),
        })
    return in_maps


def run_sharded(neighbor_vectors, neighbor_relations, user_embeddings,
                trace=False):
    """Run the SPMD kernel on all 8 cores; returns (output, BassKernelResults)."""
    from concourse.bass_utils import run_bass_kernel_spmd

    nc = _get_nc()
    in_maps = _shard_inputs(neighbor_vectors, neighbor_relations,
                            user_embeddings)
    res = run_bass_kernel_spmd(nc, in_maps, list(range(N_CORES)), trace=trace)
    outs = [res.results[c]["out"] for c in range(N_CORES)]
    return np.concatenate(outs, axis=0), res


def kernel(self_vectors=None, neighbor_vectors=None, neighbor_relations=None,
           user_embeddings=None, neighbor_size=None, **_unused):
    out, _ = run_sharded(neighbor_vectors, neighbor_relations, user_embeddings)
    return out


if __name__ == "__main__":
    rng = np.random.default_rng(0)
    nv = rng.standard_normal((B_FULL, NITER * NSIZE, DIM), dtype=np.float32)
    nr = rng.standard_normal((B_FULL, NITER * NSIZE, DIM), dtype=np.float32)
    ue = rng.standard_normal((B_FULL, DIM), dtype=np.float32)
    o = kernel(neighbor_vectors=nv, neighbor_relations=nr, user_embeddings=ue)
    print(o.shape, o.dtype)
